# revision 18
# baseline (speedup 1.0000x reference)
"""Trainium2 Bass kernel for nn_AttentiveEncoder (embed -> linear -> full self-attention).

Input-distribution-minimal sharding: the dominant cost of this problem is
moving bytes (host memcpy + PCIe), so the host routes only what each core
needs and every host-side copy is either a contiguous view or fused.

  host:  one fused gather emb_table[ids] cast to bf16 (16 MB); core c gets
         the view rows[1024c:1024c+1024]. W ships as 8 bf16 128-row
         contiguous shards (2 MB total; the kernel rounds W to bf16
         internally anyway, so this loses nothing). Output returns bf16 and
         is cast-assigned into the f32 result. Total upload ~34 MB vs the
         1.08 GB a replicated f32 embedding table would cost.
  device, phase A: AllGather the W row-shards -> full W; build the 128x128
         identity with memset+affine_select; PE-transpose E and W (W.T is
         needed k-partitioned for the linear); L = E @ W.T + b in bf16
         matmuls with f32 PSUM accumulation, written bf16 (l_bf); L^T is 16
         PE transposes of l_bf per chunk (bias inherited, K/V bit-identical).
  exchange: TWO half AllGathers (2 MB -> 16 MB each), each packing
         [512 natural L rows ; L^T half, row 512+(p*KT+kt)//2,
         col ((p*KT+kt)%2)*512+i] contiguously. The first launches while
         phase A is still computing the second half, and attention chunks
         0-1 run while the second gather is in flight - the collective
         mostly hides. Fewer, bigger collectives: measured AllGather cost is
         ~16 us/launch + bytes at a bandwidth that ramps with size.
  attention: per 2048-key chunk, kv_nat (natural) and kv_t (transposed,
         [h, j]) are loaded from the gathered DRAM (double-buffered). Per
         (q-group 256, key j-tile 128): S^T = K^T.T @ Q^T on the tensor
         engine, P = exp(S/sqrt(H)) bf16 on ACT, then out += P.T @ V and
         den += P.T @ 1 accumulated in PSUM per chunk. DVE flushes PSUM into
         SBUF accumulators; the last chunk's flush also normalizes
         (reciprocal of den) and stores bf16.

Q^T is the core's own slice of L^T (SBUF-resident; no DRAM round trip).
"""
import numpy as np
import ml_dtypes
from contextlib import ExitStack

import concourse.bass as bass
import concourse.bacc as bacc
import concourse.tile as tile
from concourse import mybir
from concourse.bass_utils import run_bass_kernel_spmd

F32 = mybir.dt.float32
F32R = mybir.dt.float32r
BF16 = mybir.dt.bfloat16

N_CORES = 8
VOCAB = 32000
H = 1024             # hidden
SEQ = 8192           # sequence
NQ = SEQ // N_CORES  # query rows per core (1024)
KT = H // 128        # k-tiles over hidden (8)
CHUNK = 256          # rows per core per attention chunk
G = NQ // CHUNK      # chunks (4)
QGS = 256            # query rows per q-group
NQG = NQ // QGS      # q-groups per core (4)
IC = QGS // 128      # i-chunks per q-group (2)
HC = H // 512        # h-chunks (2)
SCALE = 1.0 / np.sqrt(np.float32(H))

_cached = None


def _build(sim_single_core=False):
    nc = bacc.Bacc()

    e_rows = nc.dram_tensor("e_rows", [NQ, H], BF16, kind="ExternalInput")
    w_rows = nc.dram_tensor("w_rows", [128, H], BF16, kind="ExternalInput")  # W rows
    bias = nc.dram_tensor("bias", [1, H], F32, kind="ExternalInput")
    out_d = nc.dram_tensor("out", [NQ, H], BF16, kind="ExternalOutput")

    with tile.TileContext(nc) as tc, ExitStack() as ctx:
        pers = ctx.enter_context(tc.tile_pool(name="pers", bufs=1))
        dram = ctx.enter_context(tc.tile_pool(name="dram", bufs=1, space="DRAM"))

        lt_sb = pers.tile([128, KT, NQ], BF16, tag="lt_sb")   # L^T = Q^T [h, i]
        out_acc = pers.tile([128, NQ // 128, H], F32, tag="out_acc")
        den_acc = pers.tile([128, NQ // 128], F32, tag="den_acc")
        ones_bf = pers.tile([128, 1], BF16, tag="ones_bf")
        nc.vector.memset(ones_bf[:], 1.0)

        # Two exchange halves; each packs [512 natural L rows ; L^T half
        # (row 512 + (p*KT+kt)//2, col ((p*KT+kt)%2)*512 + i)] contiguously so
        # the first AllGather can launch while phase A computes the second half.
        comb_h = [dram.tile([NQ, H], BF16, name=f"comb{h}") for h in range(2)]
        w_all = dram.tile([H, H], BF16, addr_space="Shared", name="w_all")
        gath_h = [dram.tile([N_CORES * NQ, H], BF16, addr_space="Shared",
                            name=f"gath{h}") for h in range(2)]

        # ---------------- phase A ----------------
        if not sim_single_core:
            w_sstage = dram.tile([128, H], BF16, name="w_sstage")
            nc.sync.dma_start(w_sstage[:], w_rows[:])
            nc.gpsimd.collective_compute(
                "AllGather", mybir.AluOpType.bypass,
                replica_groups=[list(range(N_CORES))],
                ins=[w_sstage[:]], outs=[w_all.opt()],
            )

        with tc.tile_pool(name="pa", bufs=1) as pa:
            e_nat = pa.tile([128, NQ // 128, H], BF16, tag="e_nat")
            nc.sync.dma_start(e_nat[:], e_rows.rearrange("(a p) h -> p a h", p=128))
            # identity for PE transposes: ones + keep-diagonal
            id_sb = pa.tile([128, 128], F32, tag="id_sb")
            nc.vector.memset(id_sb[:], 1.0)
            nc.gpsimd.affine_select(
                id_sb[:], id_sb[:], pattern=[[-1, 128]],
                compare_op=mybir.AluOpType.is_equal, fill=0.0,
                base=0, channel_multiplier=1,
            )
            id_bf = pa.tile([128, 128], BF16, tag="id_bf")
            nc.vector.tensor_copy(id_bf[:], id_sb[:])
            b_sb = pa.tile([1, H], F32, tag="b_sb")
            nc.sync.dma_start(b_sb[:], bias[:])
            b_b = pa.tile([1, H], BF16, tag="b_b")
            nc.vector.tensor_copy(b_b[:], b_sb[:])
            one_b = pa.tile([1, 512], BF16, tag="one_b")
            nc.vector.memset(one_b[:], 1.0)

            # E^T via PE transposes (needed as lhsT/rhs for both L matmuls)
            e_t = pa.tile([128, KT, NQ], BF16, tag="e_t")
            with tc.tile_pool(name="pa_tpe", bufs=2, space="PSUM") as pa_tpe:
                for it in range(NQ // 128):
                    for kt in range(KT):
                        tpb = pa_tpe.tile([128, 128], BF16, tag="tpb")
                        nc.tensor.transpose(tpb[:],
                                            e_nat[:, it, kt * 128:(kt + 1) * 128],
                                            id_bf[:])
                        nc.vector.tensor_copy(e_t[:, kt, it * 128:(it + 1) * 128],
                                              tpb[:])

            # W^T via PE transposes of the AllGathered natural W
            w_nat = pa.tile([128, KT, H], BF16, tag="w_nat")
            if sim_single_core:
                for ht in range(KT):
                    nc.sync.dma_start(w_nat[:, ht, :], w_rows[:])
            else:
                nc.sync.dma_start(w_nat[:], w_all.rearrange("(ht p) k -> p ht k", p=128))
            w_b = pa.tile([128, KT, H], BF16, tag="w_b")
            with tc.tile_pool(name="pa_tpw", bufs=2, space="PSUM") as pa_tpw:
                for ht in range(KT):
                    for kt in range(KT):
                        tp = pa_tpw.tile([128, 128], BF16, tag="tp")
                        nc.tensor.transpose(tp[:],
                                            w_nat[:, ht, kt * 128:(kt + 1) * 128],
                                            id_bf[:])
                        nc.scalar.copy(w_b[:, kt, ht * 128:(ht + 1) * 128], tp[:])

            l_bf = pa.tile([128, NQ // 128, H], BF16, tag="l_bf")
            l_stage_r = [comb_h[h][0:NQ // 2, :].rearrange("(a p) h2 -> p a h2", p=128)
                         for h in range(2)]
            lt_stage_r = [comb_h[h][NQ // 2:NQ, :]
                          .rearrange("(p k4) (k2 i) -> p (k4 k2) i", p=128, k4=4, k2=2)
                          for h in range(2)]

            with tc.tile_pool(name="pa_ps", bufs=2, space="PSUM") as pa_ps, \
                 tc.tile_pool(name="pa_ps2", bufs=2, space="PSUM") as pa_ps2:
                for g in range(G):
                    # natural L for this chunk's two i-tiles
                    for half in range(2):
                        it = 2 * g + half
                        ps = pa_ps.tile([128, HC, 512], F32, tag="ps")
                        for hc in range(HC):
                            for kt in range(KT):
                                nc.tensor.matmul(
                                    ps[:, hc, :],
                                    e_t[:, kt, it * 128:(it + 1) * 128],
                                    w_b[:, kt, hc * 512:(hc + 1) * 512],
                                    start=(kt == 0), stop=False,
                                )
                            nc.tensor.matmul(
                                ps[:, hc, :], one_b[:, 0:128],
                                b_b[:, hc * 512:(hc + 1) * 512],
                                start=False, stop=True,
                            )
                            nc.scalar.copy(l_bf[:, it, hc * 512:(hc + 1) * 512],
                                           ps[:, hc, :])
                    # transposed L^T: PE-transpose the just-written l_bf
                    # tiles (bias already included; K/V become bit-identical)
                    for half in range(2):
                        it = 2 * g + half
                        for ht in range(KT):
                            tpl = pa_ps2.tile([128, 128], BF16, tag="tpl")
                            nc.tensor.transpose(
                                tpl[:], l_bf[:, it, ht * 128:(ht + 1) * 128],
                                id_bf[:])
                            nc.vector.tensor_copy(
                                lt_sb[:, ht, it * 128:(it + 1) * 128], tpl[:])
                    # stage natural chunk (ACT HWDGE queues, separate from SP loads)
                    h, sub = g // 2, g % 2
                    nc.scalar.dma_start(
                        l_stage_r[h][:, 2 * sub:2 * sub + 2, :],
                        l_bf[:, 2 * g:2 * g + 2, :])
                    if sub == 1:
                        # this half's L^T columns are complete: stage + gather
                        nc.scalar.dma_start(
                            lt_stage_r[h],
                            lt_sb[:, :, h * (NQ // 2):(h + 1) * (NQ // 2)])
                        if not sim_single_core:
                            nc.gpsimd.collective_compute(
                                "AllGather", mybir.AluOpType.bypass,
                                replica_groups=[list(range(N_CORES))],
                                ins=[comb_h[h][:]], outs=[gath_h[h].opt()],
                            )

        # ---------------- attention over gathered keys ----------------
        nblk = 1 if sim_single_core else N_CORES
        srcs = comb_h if sim_single_core else gath_h
        with tc.tile_pool(name="kv", bufs=2) as kvp, \
             tc.tile_pool(name="pt", bufs=4) as ptp, \
             tc.tile_pool(name="st_ps", bufs=2, space="PSUM") as st_ps, \
             tc.tile_pool(name="out_ps", bufs=1, space="PSUM") as out_ps, \
             tc.tile_pool(name="den_ps", bufs=1, space="PSUM") as den_ps, \
             tc.tile_pool(name="fin", bufs=2) as fin:
            for g in range(G):
                CJ = nblk * CHUNK  # keys per chunk
                kv_nat = kvp.tile([128, CJ // 128, H], BF16, tag="kv_nat",
                                  name=f"kv_nat{g}")
                kv_t = kvp.tile([128, KT, CJ], BF16, tag="kv_t", name=f"kv_t{g}")
                h, sub = g // 2, g % 2
                for r in range(nblk):
                    base = r * NQ
                    nc.sync.dma_start(
                        kv_nat[:, r * (CHUNK // 128):(r + 1) * (CHUNK // 128), :],
                        srcs[h][base + sub * CHUNK:base + (sub + 1) * CHUNK, :]
                        .rearrange("(a p) h2 -> p a h2", p=128))
                    nc.sync.dma_start(
                        kv_t[:, :, r * CHUNK:(r + 1) * CHUNK],
                        srcs[h][base + NQ // 2:base + NQ, :]
                        .rearrange("(p k4) (k2 i) -> p (k4 k2) i",
                                   p=128, k4=4, k2=2)[:, :, sub * CHUNK:(sub + 1) * CHUNK])

                for qg in range(NQG):
                    ops = out_ps.tile([128, 2 * HC, 512], F32, tag="ops")
                    dps = [den_ps.tile([128, 1], F32, tag=f"dps{ic}",
                                       name=f"dps{g}_{qg}_{ic}") for ic in range(IC)]
                    NJT = CJ // 128  # j-tiles per chunk
                    for jt in range(NJT):
                        st = st_ps.tile([128, QGS], F32, tag="st")
                        for ht in range(KT):
                            nc.tensor.matmul(
                                st[:],
                                kv_t[:, ht, jt * 128:(jt + 1) * 128],
                                lt_sb[:, ht, qg * QGS:(qg + 1) * QGS],
                                start=(ht == 0), stop=(ht == KT - 1),
                            )
                        p_t = ptp.tile([128, QGS], BF16, tag="p_t")
                        nc.scalar.activation(p_t[:], st[:],
                                             mybir.ActivationFunctionType.Exp,
                                             scale=float(SCALE))
                        first, last = (jt == 0), (jt == NJT - 1)
                        for ic in range(IC):
                            lhs = p_t[:, ic * 128:(ic + 1) * 128]
                            for hc in range(HC):
                                nc.tensor.matmul(
                                    ops[:, ic * HC + hc, :],
                                    lhs, kv_nat[:, jt, hc * 512:(hc + 1) * 512],
                                    start=first, stop=last,
                                )
                            nc.tensor.matmul(
                                dps[ic][:], lhs, ones_bf[:],
                                start=first, stop=last,
                            )
                    # flush psum accumulators into SBUF accumulators;
                    # last chunk: normalize + store immediately
                    out_r = out_d.rearrange("(a p) h -> p a h", p=128)
                    for ic in range(IC):
                        gi = qg * IC + ic
                        acc = out_acc[:, gi, :]
                        pslice = ops[:, ic * HC:(ic + 1) * HC, :]
                        if g == 0:
                            nc.vector.tensor_copy(acc, pslice.opt())
                            nc.vector.tensor_copy(den_acc[:, gi:gi + 1], dps[ic][:])
                        else:
                            nc.vector.tensor_add(acc, acc, pslice.opt())
                            nc.vector.tensor_add(den_acc[:, gi:gi + 1],
                                                 den_acc[:, gi:gi + 1], dps[ic][:])
                        if g == G - 1:
                            recip = pers.tile([128, 1], F32, tag=f"recip{gi}",
                                              name=f"recip{gi}")
                            nc.vector.reciprocal(recip[:], den_acc[:, gi:gi + 1])
                            o = fin.tile([128, H], BF16, tag="o")
                            nc.vector.tensor_scalar_mul(o[:], acc, recip[:])
                            nc.sync.dma_start(out_r[:, gi, :], o[:])

    nc.compile()
    return nc


def _get_nc():
    global _cached
    if _cached is None:
        _cached = _build()
    return _cached


last_results = None
_last_in_maps = None


def kernel(input, emb_table, W, b):
    global last_results, _last_in_maps
    nc = _get_nc()

    ids = np.asarray(input)
    emb_np = np.asarray(emb_table, dtype=np.float32)
    w_np = np.asarray(W, dtype=np.float32).astype(ml_dtypes.bfloat16)
    b_np = np.ascontiguousarray(np.asarray(b, dtype=np.float32).reshape(1, H))

    rows_all = emb_np[ids].astype(ml_dtypes.bfloat16)  # fused gather + bf16
    in_maps = []
    for c in range(N_CORES):
        in_maps.append({
            "e_rows": rows_all[c * NQ:(c + 1) * NQ],      # view
            "w_rows": w_np[c * 128:(c + 1) * 128],        # view
            "bias": b_np,
        })

    _last_in_maps = in_maps
    res = None
    for attempt in range(3):
        try:
            res = run_bass_kernel_spmd(nc, in_maps, list(range(N_CORES)))
            break
        except Exception:
            # transient device wedge (e.g. NRT_EXEC_UNIT_UNRECOVERABLE under
            # axon); back off briefly and retry before giving up
            if attempt == 2:
                raise
            import time
            time.sleep(5.0 * (attempt + 1))
    last_results = res
    out = np.empty((SEQ, H), dtype=np.float32)
    for c in range(N_CORES):
        out[c * NQ:(c + 1) * NQ] = res.results[c]["out"]  # bf16 -> f32 cast-assign
    return out
